# revision 31
# baseline (speedup 1.0000x reference)
"""Trainium2 Bass kernel for a dense transformer block.

Math (per batch element b of x[4, 2048, 768]):
    x = x + Attn(LN1(x));  x = x + MLP(LN2(x))   (12 heads, hidden 3072, exact gelu)

Sharding: 8 cores = (batch b in 0..3) x (sequence half g in 0..1). Each core
computes the full block for its own 1024 query tokens; k/v projections are
recomputed over the full 2048-token sequence of its batch element (no
collectives). Per-core token order is rotated so own tokens are always
columns 0..1023 -> one SPMD program for all cores.

Precision plan (rel-err budget 2e-2, measured ~1e-2):
  - qkv projection: fp8e4 DoubleRow matmuls. Weights split host-side into
    hi+lo fp8 pairs at scale 16 (weight error ~0.2%, better than bf16);
    activations z1 are single fp8 (the only real quantization).
  - scores q@k: fp8e4 DoubleRow. q/k stored fp8 in a head-quad layout
    (head h lives on partitions (h%4)*32..+32 of chunk pair 2(h//4)) so the
    64-dim contraction becomes 32 partitions x 2 DoubleRow slots.
  - attention A@V: fp8e4 DoubleRow. Probabilities stored as exp(s)/64 in
    fp8 (unnormalized); the ones-column denominator uses the same quantized
    values, so the softmax ratio is self-consistent. v stored fp8 direct.
  - proj, fc1, fc2: bf16 (fp8 activations there cost too much accuracy).

Schedule: LN1/qkv pipelined per 512-token group; attention runs query-group
ng0 then ng1, with ng0's whole MLP (proj/LN2/fc1/fc2) emitted into ng1's
ACT-bound softmax window; only ng1's MLP remains as the tail.

On-device layout is channel-major ("transposed"): activations live as
[d, tokens] so the contraction dim is always on SBUF partitions.
"""

import sys

import numpy as np

sys.path.insert(0, "/opt/trn_rl_repo")

import ml_dtypes  # noqa: E402

import concourse.bacc as bacc  # noqa: E402
import concourse.mybir as mybir  # noqa: E402
import concourse.tile as tile  # noqa: E402
from concourse.bass_utils import run_bass_kernel_spmd  # noqa: E402

# Route Exp/Ln/Square/Copy to the one ACT table that holds them all
# ("natural_log_exp_and_others"): the first-match table assignment would
# otherwise bounce between tables on every LN-stats / softmax boundary,
# costing a ~1.3us table reload each time. Blanking the earlier sets (ids
# preserved) makes first-match land on the shared table.
import functools  # noqa: E402

_orig_gat = bacc.get_activation_tables


@functools.cache
def _shared_act_tables(arch):
    tabs = dict(_orig_gat(arch))
    out = {}
    seen = False
    for name, funcs in tabs.items():
        if name == "natural_log_exp_and_others":
            seen = True
        out[name] = funcs if seen else set()
    # fall back untouched if the expected set is missing
    return out if seen else tabs


bacc.get_activation_tables = _shared_act_tables

F32 = mybir.dt.float32
BF16 = mybir.dt.bfloat16
FP8 = mybir.dt.float8e4
AF = mybir.ActivationFunctionType
OP = mybir.AluOpType
DR = mybir.MatmulPerfMode.DoubleRow

P = 128
D = 768
DC = D // P            # 6 chunks of the model dim
H = 12
HD = 64
HID = 3072
HC = HID // P          # 24 chunks of the mlp hidden dim
EPS = 1e-5
SCALE = HD ** -0.5
WS = 16.0              # host-side fp8 weight scale for qkv
PSC = 1.0 / 64.0       # exp(s)*PSC stored in fp8 (unnormalized probs)
VP = 68                # padded v row length (stride 12*68 % 16 == 0)

NB = 4                 # batch
NT = 2048              # tokens per batch element (keys)
NO = NT // 2           # own tokens per core (queries)
N_CORES = 8

NPBF16 = ml_dtypes.bfloat16
NPFP8 = ml_dtypes.float8_e4m3


def _build_nc(nt, no, with_qk_bias, with_fc2_bias, reps=1, upto=99):
    """Build + schedule the SPMD Bass program (one core's view)."""
    nc = bacc.Bacc("TRN2", target_bir_lowering=False, debug=False,
                   num_devices=N_CORES)

    io = dict(
        xT=nc.dram_tensor("xT", [P, DC, nt], BF16, kind="ExternalInput"),
        wqk_hi=nc.dram_tensor("wqk_hi", [P, DC, 2 * D], FP8,
                              kind="ExternalInput"),
        wqk_lo=nc.dram_tensor("wqk_lo", [P, DC, 2 * D], FP8,
                              kind="ExternalInput"),
        wv_hi=nc.dram_tensor("wv_hi", [P, DC, D], FP8, kind="ExternalInput"),
        wv_lo=nc.dram_tensor("wv_lo", [P, DC, D], FP8, kind="ExternalInput"),
        pwT=nc.dram_tensor("pwT", [P, DC, D], BF16, kind="ExternalInput"),
        w1T=nc.dram_tensor("w1T", [P, DC, HID], BF16, kind="ExternalInput"),
        w2T=nc.dram_tensor("w2T", [P, HC, D], BF16, kind="ExternalInput"),
        qk_bias=nc.dram_tensor("qk_bias", [P, 2 * DC], F32,
                               kind="ExternalInput"),
        pb=nc.dram_tensor("pb", [P, DC], F32, kind="ExternalInput"),
        b1p=nc.dram_tensor("b1p", [P, HC], F32, kind="ExternalInput"),
        fc2_b=nc.dram_tensor("fc2_b", [P, DC], F32, kind="ExternalInput"),
        outT=nc.dram_tensor("outT", [P, DC, no], F32, kind="ExternalOutput"),
    )

    with tile.TileContext(nc) as tc:
        for _ in range(reps):
            _emit(tc, nc, io, nt, no, with_qk_bias, with_fc2_bias, upto)

    nc.compile()
    return nc


def _emit(tc, nc, io, nt, no, with_qk_bias, with_fc2_bias, upto=99):
    dc, hc = DC, HC
    ngk = nt // 512
    ngq = no // 512
    mt_n = nt // P
    _stack = []

    def _pool(*a, **k):
        p = tc.alloc_tile_pool(*a, **k)
        _stack.append(p)
        return p

    def _rel(p):
        assert _stack[-1] is p
        _stack.pop()
        p.release()

    def _cut():
        for p in reversed(_stack):
            p.release()
        _stack.clear()

    # ---- long-lived pools ---------------------------------------------------
    consts = _pool(name="consts", bufs=1)
    tmps = _pool(name="tmps", bufs=2)
    ps_mm = _pool(name="ps_mm", bufs=2, space="PSUM")
    p_x1 = _pool(name="p_x1", bufs=1)
    p_xown = _pool(name="p_xown", bufs=1)
    p_pw = _pool(name="p_pw", bufs=1)
    p_w1 = _pool(name="p_w1", bufs=3)       # streamed fc1 weight slices
    p_w2 = _pool(name="p_w2", bufs=3)       # streamed fc2 weight slices
    p_attn1 = _pool(name="p_attn1", bufs=1)

    ones_sb = consts.tile([P, P], BF16)
    nc.vector.memset(ones_sb, 1.0)
    qkb_sb = consts.tile([P, 2 * dc], F32)
    nc.sync.dma_start(qkb_sb, io["qk_bias"][:, :])
    pb_sb = consts.tile([P, dc], F32)
    nc.sync.dma_start(pb_sb, io["pb"][:, :])
    b1p_sb = consts.tile([P, hc], F32)
    nc.sync.dma_start(b1p_sb, io["b1p"][:, :])
    fc2b_sb = consts.tile([P, dc], F32)
    nc.sync.dma_start(fc2b_sb, io["fc2_b"][:, :])
    eps_sb = consts.tile([P, 1], F32)
    nc.vector.memset(eps_sb, EPS)
    zero_sb = consts.tile([P, 1], F32)
    nc.vector.memset(zero_sb, 0.0)
    lpsc_sb = consts.tile([P, 1], F32)
    nc.vector.memset(lpsc_sb, float(np.log(PSC)))

    pw_sb = p_pw.tile([P, dc, D], BF16)

    x1T = p_x1.tile([P, dc, no], F32)
    xown = p_xown.tile([P, dc, no], BF16)
    attn1 = p_attn1.tile([P, dc, 512], BF16)

    p_qkvout = _pool(name="p_qkvout", bufs=1)
    q8 = p_qkvout.tile([P, dc, no], FP8, tag="q")
    k8 = p_qkvout.tile([P, dc, nt], FP8, tag="k")
    v_ext = p_qkvout.tile([P, mt_n, H, VP], FP8, tag="v")
    nc.vector.memset(v_ext[:, :, :, HD:HD + 1], 1.0)

    # ------- Phase 1+2: per-512-group LN1 stats -> z1 -> q/k/v projections --
    # First tokens first: the group-0 x load gates the whole pipeline, so it
    # is issued before any weight DMA.
    xT_t = io["xT"][:, :, :]
    nc.sync.dma_start(xown[:, :, 0:512], xT_t[:, :, 0:512])

    p_wqkv = _pool(name="p_wqkv", bufs=1)
    wqk_hi = p_wqkv.tile([P, dc, 2 * D], FP8, tag="wqk_hi")
    nc.sync.dma_start(wqk_hi, io["wqk_hi"][:, :, :])
    wqk_lo = p_wqkv.tile([P, dc, 2 * D], FP8, tag="wqk_lo")
    nc.sync.dma_start(wqk_lo, io["wqk_lo"][:, :, :])
    wv_hi = p_wqkv.tile([P, dc, D], FP8, tag="wv_hi")
    nc.sync.dma_start(wv_hi, io["wv_hi"][:, :, :])
    wv_lo = p_wqkv.tile([P, dc, D], FP8, tag="wv_lo")
    nc.sync.dma_start(wv_lo, io["wv_lo"][:, :, :])
    w8 = (wqk_hi, wqk_lo)
    wv8 = (wv_hi, wv_lo)

    p_z1g = _pool(name="p_z1g", bufs=2)
    p_xg = _pool(name="p_xg", bufs=2)
    p_stat = _pool(name="p_stat", bufs=2)
    p_scr = _pool(name="p_scr", bufs=2)
    ps_st = _pool(name="ps_st", bufs=2, space="PSUM")

    for g in range(ngk):
        gsl = slice(g * 512, (g + 1) * 512)
        if g < ngq:
            xg = xown[:, :, gsl]
            if g > 0:
                nc.sync.dma_start(xg, xT_t[:, :, gsl])
        else:
            xg = p_xg.tile([P, dc, 512], BF16, tag="xg", name=f"xg_{g}")
            nc.sync.dma_start(xg, xT_t[:, :, gsl])

        nm, rs = _group_stats(nc, ps_st, p_stat, p_scr, tmps, ones_sb,
                              eps_sb, zero_sb, xg, dc, name=f"s1_{g}")
        z1g = p_z1g.tile([P, dc, 512], FP8, tag="z1", name=f"z1_{g}")
        for c in range(dc):
            # all-bf16 add runs in the DVE 2x mode; fp8 quant dominates error
            t = tmps.tile([P, 512], BF16, tag="lnt1")
            nc.vector.tensor_add(t, xg[:, c], nm)
            nc.vector.tensor_mul(z1g[:, c], t, rs)

        # q/k for this token group (k for all groups, q for own groups)
        for cc in range(2 * dc):
            is_q = cc < dc
            if is_q and g >= ngq:
                continue
            msl = slice(cc * P, (cc + 1) * P)
            ps = ps_mm.tile([P, 512], F32, tag="mm")
            for hl in range(2):
                for j in range(dc // 2):
                    nc.tensor.matmul(
                        ps, w8[hl][:, 2 * j:2 * j + 2, msl],
                        z1g[:, 2 * j:2 * j + 2, :],
                        start=(hl == 0 and j == 0),
                        stop=(hl == 1 and j == dc // 2 - 1),
                        perf_mode=DR)
            dst = q8 if is_q else k8
            dcc = cc if is_q else cc - dc
            if with_qk_bias:
                nc.scalar.activation(dst[:, dcc, gsl], ps, AF.Identity,
                                     bias=qkb_sb[:, cc:cc + 1],
                                     scale=1.0 / WS)
            else:
                nc.scalar.mul(dst[:, dcc, gsl], ps, 1.0 / WS)

        # v for the 4 token blocks of this group
        for mt in range(4 * g, 4 * g + 4):
            lsl = slice((mt - 4 * g) * P, (mt - 4 * g + 1) * P)
            for half in range(2):
                rhs_sl = slice(half * 384, (half + 1) * 384)
                ps = ps_mm.tile([P, 384], F32, tag="mm")
                for hl in range(2):
                    for j in range(dc // 2):
                        nc.tensor.matmul(
                            ps, z1g[:, 2 * j:2 * j + 2, lsl],
                            wv8[hl][:, 2 * j:2 * j + 2, rhs_sl],
                            start=(hl == 0 and j == 0),
                            stop=(hl == 1 and j == dc // 2 - 1),
                            perf_mode=DR)
                dst = v_ext[:, mt, half * 6:(half + 1) * 6, 0:HD]
                nc.vector.tensor_scalar_mul(
                    dst, ps.rearrange("p (h d) -> p h d", d=HD), 1.0 / WS)

    _rel(ps_st)
    _rel(p_scr)
    _rel(p_stat)
    _rel(p_xg)
    _rel(p_z1g)
    _rel(p_wqkv)
    nc.sync.dma_start(pw_sb, io["pwT"][:, :, :])
    if upto <= 2:
        _cut()
        return

    # ---------------- Phase 3: attention + interleaved MLP(ng0) -------------
    p_win = _pool(name="p_win", bufs=1)     # ng0-scoped mlp buffers
    attn0 = p_win.tile([P, dc, 512], BF16, tag="attn0")
    z2g0 = p_win.tile([P, dc, 512], BF16, tag="z2g0")
    hT0 = p_win.tile([P, hc, 512], BF16, tag="hT0")
    p_stat2 = _pool(name="p_stat2", bufs=2)
    p_x2 = _pool(name="p_x2", bufs=3)
    ps_sc = _pool(name="ps_sc", bufs=2, space="PSUM")
    ps_av = _pool(name="ps_av", bufs=2, space="PSUM")
    p_pT = _pool(name="p_pT", bufs=3)
    p_pair = _pool(name="p_pair", bufs=2)

    w1_t = io["w1T"][:, :, :]
    w2_t = io["w2T"][:, :, :]
    outT_t = io["outT"][:, :, :]
    w1_sl = {}
    w2_sl = {}

    def w1_slice(i):
        if i not in w1_sl:
            w = p_w1.tile([P, dc, 512], BF16, tag="w1", name=f"w1_{i}")
            nc.sync.dma_start(w, w1_t[:, :, i * 512:(i + 1) * 512])
            w1_sl[i] = w
        return w1_sl[i]

    def w2_slice(i):
        if i not in w2_sl:
            w = p_w2.tile([P, hc, P], BF16, tag="w2", name=f"w2_{i}")
            nc.sync.dma_start(w, w2_t[:, :, i * P:(i + 1) * P])
            w2_sl[i] = w
        return w2_sl[i]

    def mlp_steps(ng, attn_t, z2g, hTg, wtag):
        """Generate the MLP step closures for one 512-query group."""
        sl = slice(ng * 512, (ng + 1) * 512)

        def proj_step(ec0):
            for ec in range(ec0, ec0 + 2):
                ps = ps_mm.tile([P, 512], F32, tag="mm")
                for c in range(dc):
                    nc.tensor.matmul(ps, pw_sb[:, c, ec * P:(ec + 1) * P],
                                     attn_t[:, c], start=(c == 0),
                                     stop=(c == dc - 1))
                # x1 = proj + x + pb   (residual built on the fly)
                nc.vector.scalar_tensor_tensor(
                    x1T[:, ec, sl], ps, pb_sb[:, ec:ec + 1],
                    xown[:, ec, sl], OP.add, OP.add)

        def ln2_step():
            nm, rs = _group_stats(nc, ps_mm, p_stat2, p_win, tmps, ones_sb,
                                  eps_sb, zero_sb, x1T[:, :, sl], dc,
                                  name=f"s2_{wtag}", xb=z2g)
            for c in range(dc):
                t = tmps.tile([P, 512], F32, tag="lnt")
                nc.vector.tensor_add(t, x1T[:, c, sl], nm)
                nc.vector.tensor_mul(z2g[:, c], t, rs)

        def fc1_step(cc0, n):
            # stage bias-added pre-activation on DVE, then one big in-place
            # gelu: a single ACT instruction can't be interleaved with
            # softmax exps, so the gelu table loads stay rare.
            for cc in range(cc0, cc0 + n):
                w = w1_slice(cc // 4)
                ci = cc % 4
                ps = ps_mm.tile([P, 512], F32, tag="mm")
                for c in range(dc):
                    nc.tensor.matmul(ps, w[:, c, ci * P:(ci + 1) * P],
                                     z2g[:, c], start=(c == 0),
                                     stop=(c == dc - 1))
                nc.vector.tensor_scalar(hTg[:, cc], ps,
                                        b1p_sb[:, cc:cc + 1], None, OP.add)
            nc.scalar.activation(hTg[:, cc0:cc0 + n], hTg[:, cc0:cc0 + n],
                                 AF.Gelu, bias=zero_sb[:, 0:1])

        def fc2_step(ec0, n):
            for ec in range(ec0, ec0 + n):
                w = w2_slice(ec)
                ps = ps_mm.tile([P, 512], F32, tag="mm")
                for c in range(hc):
                    nc.tensor.matmul(ps, w[:, c], hTg[:, c],
                                     start=(c == 0), stop=(c == hc - 1))
                x2 = p_x2.tile([P, 512], F32, tag="x2", bufs=3)
                if with_fc2_bias:
                    nc.vector.scalar_tensor_tensor(
                        x2, ps, fc2b_sb[:, ec:ec + 1], x1T[:, ec, sl],
                        OP.add, OP.add)
                else:
                    nc.vector.tensor_add(x2, ps, x1T[:, ec, sl])
                nc.sync.dma_start(outT_t[:, ec, sl], x2)

        # fc1 in two 12-wide chunks keeps the gelu ops contiguous on ACT
        # (fewer activation-table reloads against softmax's exp).
        return ([lambda e=e: proj_step(e) for e in (0, 2, 4)]
                + [ln2_step]
                + [lambda c=c: fc1_step(c, 12) for c in (0, 12)]
                + [lambda e=e: fc2_step(e, 3) for e in (0, 3)])

    steps0 = mlp_steps(0, attn0, z2g0, hT0, "a")

    for ng in range(ngq):
        sl = slice(ng * 512, (ng + 1) * 512)
        attn_t = attn0 if ng == 0 else attn1
        pair_sb = None
        bc = None
        for h in range(H):
            g4, slot = divmod(h, 4)
            prange = slice(slot * 32, (slot + 1) * 32)
            csl = slice(2 * g4, 2 * g4 + 2)
            po = ps_av.tile([P, 512], F32, tag="av", name=f"po_{ng}_{h}")
            for t in range(mt_n // 2):
                ps_s = ps_sc.tile([P, 2, 512], F32, tag="sc")
                pp = p_pT.tile([P, 2, 512], FP8, tag="pT", bufs=3)
                for i in range(2):
                    mt = 2 * t + i
                    nc.tensor.matmul(ps_s[:, i],
                                     k8[prange, csl, mt * P:(mt + 1) * P],
                                     q8[prange, csl, sl], perf_mode=DR,
                                     tile_position=(slot * 32, 0))
                # p = exp(s * SCALE) * PSC, fp8 unnormalized
                nc.scalar.activation(pp, ps_s, AF.Exp,
                                     bias=lpsc_sb[:, 0:1], scale=SCALE)
                nc.tensor.matmul(po[0:HD + 1],
                                 v_ext[:, 2 * t:2 * t + 2, h, 0:HD + 1],
                                 pp, start=(t == 0), stop=(t == mt_n // 2 - 1),
                                 perf_mode=DR)
            # evacuate + normalize per head pair
            hp, sub = divmod(h, 2)
            if sub == 0:
                pair_sb = p_pair.tile([P, 512], F32, tag="pair",
                                      name=f"pair_{ng}_{hp}")
                bc = ps_av.tile([P, 512], F32, tag="av", name=f"bc_{ng}_{hp}")
            nc.vector.tensor_copy(pair_sb[sub * HD:(sub + 1) * HD], po[0:HD])
            rec_h = tmps.tile([1, 512], BF16, tag="rec", bufs=4)
            with nc.allow_low_precision(reason="softmax denom recip"):
                nc.vector.reciprocal(rec_h, po[HD:HD + 1])
            nc.tensor.matmul(bc[sub * HD:(sub + 1) * HD],
                             ones_sb[0:1, 0:HD], rec_h)
            if sub == 1:
                nc.vector.tensor_mul(attn_t[:, hp], pair_sb, bc)
            # interleave ng0's MLP into ng1's softmax window
            if ng == 1 and h < len(steps0):
                steps0[h]()
        if ng == 1:
            for s in steps0[H:]:
                s()

    _rel(p_pair)
    _rel(p_pT)
    _rel(ps_av)
    _rel(ps_sc)
    _rel(p_x2)
    _rel(p_stat2)
    _rel(p_win)
    _rel(p_qkvout)
    if upto <= 3:
        _cut()
        return

    # ---------------- Tail: MLP for ng1 -------------------------------------
    p_tail = _pool(name="p_tail", bufs=1)
    z2g1 = p_tail.tile([P, dc, 512], BF16, tag="z2g1")
    hT1 = p_tail.tile([P, hc, 512], BF16, tag="hT1")
    p_stat3 = _pool(name="p_stat3", bufs=2)
    p_x2b = _pool(name="p_x2b", bufs=3)

    # streamed weight slices were rotated during the window; reload fresh
    w1_sl.clear()
    w2_sl.clear()
    steps1 = _tail_steps(nc, ps_mm, p_stat3, p_tail, p_x2b, tmps, ones_sb,
                         eps_sb, zero_sb, pw_sb, pb_sb, b1p_sb, fc2b_sb,
                         x1T, xown, attn1, z2g1, hT1, w1_slice, w2_slice,
                         outT_t, dc, hc, with_fc2_bias)
    for s in steps1:
        s()

    _rel(p_x2b)
    _rel(p_stat3)
    _rel(p_tail)
    _cut()


def _tail_steps(nc, ps_mm, p_stat, p_win, p_x2, tmps, ones_sb, eps_sb,
                zero_sb, pw_sb, pb_sb, b1p_sb, fc2b_sb, x1T, xown, attn_t,
                z2g, hTg, w1_slice, w2_slice, outT_t, dc, hc, with_fc2_bias):
    sl = slice(512, 1024)
    steps = []

    def proj_step(ec0):
        for ec in range(ec0, ec0 + 2):
            ps = ps_mm.tile([P, 512], F32, tag="mm")
            for c in range(dc):
                nc.tensor.matmul(ps, pw_sb[:, c, ec * P:(ec + 1) * P],
                                 attn_t[:, c], start=(c == 0),
                                 stop=(c == dc - 1))
            nc.vector.scalar_tensor_tensor(
                x1T[:, ec, sl], ps, pb_sb[:, ec:ec + 1],
                xown[:, ec, sl], OP.add, OP.add)

    def ln2_step():
        nm, rs = _group_stats(nc, ps_mm, p_stat, p_win, tmps, ones_sb,
                              eps_sb, zero_sb, x1T[:, :, sl], dc,
                              name="s2_b", xb=z2g)
        for c in range(dc):
            t = tmps.tile([P, 512], F32, tag="lnt")
            nc.vector.tensor_add(t, x1T[:, c, sl], nm)
            nc.vector.tensor_mul(z2g[:, c], t, rs)

    def fc1_step(cc0, n):
        for cc in range(cc0, cc0 + n):
            w = w1_slice(cc // 4)
            ci = cc % 4
            ps = ps_mm.tile([P, 512], F32, tag="mm")
            for c in range(dc):
                nc.tensor.matmul(ps, w[:, c, ci * P:(ci + 1) * P],
                                 z2g[:, c], start=(c == 0),
                                 stop=(c == dc - 1))
            nc.vector.tensor_scalar(hTg[:, cc], ps,
                                    b1p_sb[:, cc:cc + 1], None, OP.add)
        nc.scalar.activation(hTg[:, cc0:cc0 + n], hTg[:, cc0:cc0 + n],
                             AF.Gelu, bias=zero_sb[:, 0:1])

    def fc2_step(ec0, n):
        for ec in range(ec0, ec0 + n):
            w = w2_slice(ec)
            ps = ps_mm.tile([P, 512], F32, tag="mm")
            for c in range(hc):
                nc.tensor.matmul(ps, w[:, c], hTg[:, c],
                                 start=(c == 0), stop=(c == hc - 1))
            x2 = p_x2.tile([P, 512], F32, tag="x2", bufs=3)
            if with_fc2_bias:
                nc.vector.scalar_tensor_tensor(
                    x2, ps, fc2b_sb[:, ec:ec + 1], x1T[:, ec, sl],
                    OP.add, OP.add)
            else:
                nc.vector.tensor_add(x2, ps, x1T[:, ec, sl])
            nc.sync.dma_start(outT_t[:, ec, sl], x2)

    steps += [lambda e=e: proj_step(e) for e in (0, 2, 4)]
    steps.append(ln2_step)
    steps += [lambda c=c: fc1_step(c, 12) for c in (0, 12)]
    steps += [lambda e=e: fc2_step(e, 3) for e in (0, 3)]
    return steps


def _group_stats(nc, ps_pool, p_stat, p_sq, tmps, ones_sb, eps_sb, zero_sb,
                 x_g, dc, name, xb=None):
    """-mean and rstd (replicated over partitions) for one 512-token group.
    x_g: [P, dc, 512] bf16 or f32. For f32, a bf16 staging copy (into xb if
    given) feeds the token-sum matmul at 1 cyc/row."""
    is_f32 = x_g.dtype == F32
    # bf16 stats keep the z1/z2 elementwise chain in DVE 2x mode
    nm = p_stat.tile([P, 512], BF16, tag="nm", name=f"nm_{name}")
    rstd = p_stat.tile([P, 512], BF16, tag="rstd", name=f"rs_{name}")
    xsq = p_sq.tile([P, dc, 512], BF16, tag="xsq", name=f"xsq_{name}")
    for c in range(dc):
        nc.scalar.activation(xsq[:, c], x_g[:, c], AF.Square,
                             bias=zero_sb[:, 0:1])
    if is_f32:
        assert xb is not None
        for c in range(dc):
            nc.vector.tensor_copy(xb[:, c], x_g[:, c])
        xs = xb
    else:
        xs = x_g
    ps_s = ps_pool.tile([P, 512], F32, tag="mm")
    for c in range(dc):
        nc.tensor.matmul(ps_s, ones_sb, xs[:, c],
                         start=(c == 0), stop=(c == dc - 1))
    nc.vector.tensor_scalar_mul(nm, ps_s, -1.0 / D)
    ps_q = ps_pool.tile([P, 512], F32, tag="mm")
    for c in range(dc):
        nc.tensor.matmul(ps_q, ones_sb, xsq[:, c],
                         start=(c == 0), stop=(c == dc - 1))
    # var = E[x^2] - mean^2 ; rstd = exp(-ln(var + eps)/2). The ln/exp pair
    # lives in the same ACT table as softmax's exp -> no table reloads.
    var = tmps.tile([P, 512], F32, tag="var", bufs=1)
    nc.vector.tensor_scalar_mul(var, ps_q, 1.0 / D)
    msq = tmps.tile([P, 512], F32, tag="msq", bufs=1)
    nc.vector.tensor_mul(msq, nm, nm)
    nc.vector.tensor_tensor(var, var, msq, OP.subtract)
    lv = tmps.tile([P, 512], F32, tag="lv", bufs=1)
    nc.scalar.activation(lv, var, AF.Ln, bias=eps_sb[:, 0:1])
    nc.scalar.activation(rstd, lv, AF.Exp, bias=zero_sb[:, 0:1], scale=-0.5)
    return nm, rstd


# --------------------------------------------------------------------------
# Host side
# --------------------------------------------------------------------------

_NC_CACHE = {}


def _get_nc(nt, no, with_qk_bias, with_fc2_bias, reps=1, upto=99):
    key = (nt, no, with_qk_bias, with_fc2_bias, reps, upto)
    if key not in _NC_CACHE:
        _NC_CACHE[key] = _build_nc(nt, no, with_qk_bias, with_fc2_bias, reps,
                                   upto)
    return _NC_CACHE[key]


def _fp8_split(a, s):
    """a: f32 array -> (hi, lo) fp8 pair with hi + lo ~= a * s."""
    hi = np.clip(a * s, -240.0, 240.0).astype(NPFP8)
    lo = np.clip(a * s - hi.astype(np.float32), -240.0, 240.0).astype(NPFP8)
    return hi, lo


def _qk_perm():
    """Head-quad row order: chunk 2g+j holds heads 4g..4g+3, dims j*32..+32,
    with head h on partitions (h%4)*32..(h%4+1)*32."""
    perm = np.empty(2 * D, dtype=np.int64)
    for c in range(DC):
        g, j = divmod(c, 2)
        for p in range(P):
            head = 4 * g + p // 32
            dim = j * 32 + p % 32
            perm[c * P + p] = head * HD + dim
    perm[D:] = perm[:D] + D
    return perm


def _prep_weights(ln1_w, ln1_b, qkv_w, qkv_b, proj_w, proj_b,
                  ln2_w, ln2_b, fc1_w, fc1_b, fc2_w, fc2_b):
    w_qkv = qkv_w * ln1_w[None, :]
    b_qkv = qkv_w @ ln1_b + qkv_b
    pb = proj_b + proj_w @ b_qkv[2 * D:]
    w1 = fc1_w * ln2_w[None, :]
    b1p = fc1_b + fc1_w @ ln2_b

    perm = _qk_perm()
    wqk = w_qkv[:2 * D][perm]
    qkb = b_qkv[:2 * D][perm]

    def col(v, chunks):
        return np.ascontiguousarray(v.reshape(chunks, P).T.astype(np.float32))

    def sb(wT, chunks):
        # [K, M] -> [P, chunks, M] with K = chunks*P (SBUF layout)
        k, m = wT.shape
        return np.ascontiguousarray(
            wT.reshape(chunks, P, m).transpose(1, 0, 2).astype(np.float32))

    wqk_hi, wqk_lo = _fp8_split(sb(wqk.T, DC), WS)
    wv_hi, wv_lo = _fp8_split(sb(w_qkv[2 * D:].T, DC), WS)
    shared = {
        "wqk_hi": wqk_hi, "wqk_lo": wqk_lo,
        "wv_hi": wv_hi, "wv_lo": wv_lo,
        "pwT": sb(proj_w.T, DC).astype(NPBF16),
        "w1T": sb(w1.T, DC).astype(NPBF16),
        "w2T": sb(fc2_w.T, HC).astype(NPBF16),
        "qk_bias": col(qkb, 2 * DC),
        "pb": col(pb, DC),
        "b1p": col(b1p, HC),
        "fc2_b": col(fc2_b, DC),
    }
    flags = (bool(np.any(b_qkv[:2 * D])), bool(np.any(fc2_b)))
    return shared, flags


def build_in_maps(inputs):
    x = np.asarray(inputs["x"], dtype=np.float32)
    args = {k: np.asarray(v, dtype=np.float32) for k, v in inputs.items()
            if k != "x"}
    shared, (f_qk, f_f2) = _prep_weights(
        args["ln1_w"], args["ln1_b"], args["qkv_w"], args["qkv_b"],
        args["proj_w"], args["proj_b"], args["ln2_w"], args["ln2_b"],
        args["fc1_w"], args["fc1_b"], args["fc2_w"], args["fc2_b"])
    no = x.shape[1] // 2
    in_maps = []
    for core in range(N_CORES):
        b, g = divmod(core, 2)
        xr = np.roll(x[b], -g * no, axis=0)
        m = dict(shared)
        m["xT"] = np.ascontiguousarray(
            xr.T.reshape(DC, P, x.shape[1]).transpose(1, 0, 2)).astype(NPBF16)
        in_maps.append(m)
    return in_maps, (f_qk, f_f2)


def run_on_device(inputs, trace=False):
    x = np.asarray(inputs["x"], dtype=np.float32)
    nb, nt, d = x.shape
    no = nt // 2
    in_maps, (f_qk, f_f2) = build_in_maps(inputs)
    nc = _get_nc(nt, no, f_qk, f_f2)

    res = run_bass_kernel_spmd(nc, in_maps, core_ids=list(range(N_CORES)),
                               trace=trace)
    out = np.empty((nb, nt, d), dtype=np.float32)
    for core in range(N_CORES):
        b, g = divmod(core, 2)
        o = res.results[core]["outT"]          # [P, DC, no]
        out[b, g * no:(g + 1) * no, :] = o.transpose(1, 0, 2).reshape(d, no).T
    return out, res


def kernel(**inputs) -> np.ndarray:
    out, _ = run_on_device(inputs, trace=False)
    return out


# revision 38
# speedup vs baseline: 5.3733x; 5.3733x over previous
"""Trainium2 Bass kernel for a dense transformer block.

Math (per batch element b of x[4, 2048, 768]):
    x = x + Attn(LN1(x));  x = x + MLP(LN2(x))   (12 heads, hidden 3072, exact gelu)

Sharding: 8 cores = (batch b in 0..3) x (sequence half g in 0..1). Each core
computes the full block for its own 1024 query tokens; k/v projections are
recomputed over the full 2048-token sequence of its batch element (no
collectives). Per-core token order is rotated so own tokens are always
columns 0..1023 -> one SPMD program for all cores.

Precision plan (rel-err budget 2e-2, measured ~1e-2):
  - qkv projection: fp8e4 DoubleRow matmuls. Weights split host-side into
    hi+lo fp8 pairs at scale 16 (weight error ~0.2%, better than bf16);
    activations z1 are single fp8 (the only real quantization).
  - scores q@k: fp8e4 DoubleRow. q/k stored fp8 in a head-quad layout
    (head h lives on partitions (h%4)*32..+32 of chunk pair 2(h//4)) so the
    64-dim contraction becomes 32 partitions x 2 DoubleRow slots.
  - attention A@V: fp8e4 DoubleRow. Probabilities stored as exp(s)/64 in
    fp8 (unnormalized); the ones-column denominator uses the same quantized
    values, so the softmax ratio is self-consistent. v stored fp8 direct.
  - proj, fc1, fc2: bf16 (fp8 activations there cost too much accuracy).

Schedule: LN1/qkv pipelined per 512-token group; attention runs query-group
ng0 then ng1, with ng0's whole MLP (proj/LN2/fc1/fc2) emitted into ng1's
ACT-bound softmax window; only ng1's MLP remains as the tail.

On-device layout is channel-major ("transposed"): activations live as
[d, tokens] so the contraction dim is always on SBUF partitions.
"""

import sys

import numpy as np

sys.path.insert(0, "/opt/trn_rl_repo")

import ml_dtypes  # noqa: E402

import concourse.bacc as bacc  # noqa: E402
import concourse.mybir as mybir  # noqa: E402
import concourse.tile as tile  # noqa: E402
from concourse.bass_utils import run_bass_kernel_spmd  # noqa: E402

# Route Exp/Ln/Square/Copy to the one ACT table that holds them all
# ("natural_log_exp_and_others"): the first-match table assignment would
# otherwise bounce between tables on every LN-stats / softmax boundary,
# costing a ~1.3us table reload each time. Blanking the earlier sets (ids
# preserved) makes first-match land on the shared table.
import functools  # noqa: E402

_orig_gat = bacc.get_activation_tables


@functools.cache
def _shared_act_tables(arch):
    tabs = dict(_orig_gat(arch))
    out = {}
    seen = False
    for name, funcs in tabs.items():
        if name == "natural_log_exp_and_others":
            seen = True
        out[name] = funcs if seen else set()
    # fall back untouched if the expected set is missing
    return out if seen else tabs


bacc.get_activation_tables = _shared_act_tables

F32 = mybir.dt.float32
BF16 = mybir.dt.bfloat16
FP8 = mybir.dt.float8e4
AF = mybir.ActivationFunctionType
OP = mybir.AluOpType
DR = mybir.MatmulPerfMode.DoubleRow

P = 128
D = 768
DC = D // P            # 6 chunks of the model dim
H = 12
HD = 64
HID = 3072
HC = HID // P          # 24 chunks of the mlp hidden dim
EPS = 1e-5
SCALE = HD ** -0.5
WS = 16.0              # host-side fp8 weight scale for qkv
PSC = 1.0 / 64.0       # exp(s)*PSC stored in fp8 (unnormalized probs)
VP = 68                # padded v row length (stride 12*68 % 16 == 0)

NB = 4                 # batch
NT = 2048              # tokens per batch element (keys)
NO = NT // 2           # own tokens per core (queries)
N_CORES = 8

NPBF16 = ml_dtypes.bfloat16
NPFP8 = ml_dtypes.float8_e4m3


def _build_nc(nt, no, with_qk_bias, with_fc2_bias, reps=1, upto=99):
    """Build + schedule the SPMD Bass program (one core's view)."""
    nc = bacc.Bacc("TRN2", target_bir_lowering=False, debug=False,
                   num_devices=N_CORES)

    io = dict(
        xT=nc.dram_tensor("xT", [P, DC, nt], BF16, kind="ExternalInput"),
        wqk_hi=nc.dram_tensor("wqk_hi", [P, DC, 2 * D], FP8,
                              kind="ExternalInput"),
        wqk_lo=nc.dram_tensor("wqk_lo", [P, DC, 2 * D], FP8,
                              kind="ExternalInput"),
        wv_hi=nc.dram_tensor("wv_hi", [P, DC, D], FP8, kind="ExternalInput"),
        wv_lo=nc.dram_tensor("wv_lo", [P, DC, D], FP8, kind="ExternalInput"),
        pwT=nc.dram_tensor("pwT", [P, DC, D], BF16, kind="ExternalInput"),
        w1T=nc.dram_tensor("w1T", [P, DC, HID], BF16, kind="ExternalInput"),
        w2T=nc.dram_tensor("w2T", [P, HC, D], BF16, kind="ExternalInput"),
        qk_bias=nc.dram_tensor("qk_bias", [P, 2 * DC], F32,
                               kind="ExternalInput"),
        pb=nc.dram_tensor("pb", [P, DC], F32, kind="ExternalInput"),
        b1p=nc.dram_tensor("b1p", [P, HC], F32, kind="ExternalInput"),
        fc2_b=nc.dram_tensor("fc2_b", [P, DC], F32, kind="ExternalInput"),
        outT=nc.dram_tensor("outT", [P, DC, no], F32, kind="ExternalOutput"),
    )

    with tile.TileContext(nc) as tc:
        for _ in range(reps):
            _emit(tc, nc, io, nt, no, with_qk_bias, with_fc2_bias, upto)

    nc.compile()
    return nc


def _emit(tc, nc, io, nt, no, with_qk_bias, with_fc2_bias, upto=99):
    dc, hc = DC, HC
    ngk = nt // 512
    ngq = no // 512
    mt_n = nt // P
    _stack = []

    def _pool(*a, **k):
        p = tc.alloc_tile_pool(*a, **k)
        _stack.append(p)
        return p

    def _rel(p):
        assert _stack[-1] is p
        _stack.pop()
        p.release()

    def _cut():
        for p in reversed(_stack):
            p.release()
        _stack.clear()

    # ---- long-lived pools ---------------------------------------------------
    consts = _pool(name="consts", bufs=1)
    tmps = _pool(name="tmps", bufs=2)
    ps_mm = _pool(name="ps_mm", bufs=2, space="PSUM")
    p_x1 = _pool(name="p_x1", bufs=1)
    p_xown = _pool(name="p_xown", bufs=1)
    p_pw = _pool(name="p_pw", bufs=1)
    p_w1 = _pool(name="p_w1", bufs=3)       # streamed fc1 weight slices
    p_w2 = _pool(name="p_w2", bufs=2)       # streamed fc2 weight slices
    p_attn1 = _pool(name="p_attn1", bufs=1)

    ones_sb = consts.tile([P, P], BF16)
    nc.vector.memset(ones_sb, 1.0)
    qkb_sb = consts.tile([P, 2 * dc], F32)
    nc.sync.dma_start(qkb_sb, io["qk_bias"][:, :])
    pb_sb = consts.tile([P, dc], F32)
    nc.sync.dma_start(pb_sb, io["pb"][:, :])
    b1p_sb = consts.tile([P, hc], F32)
    nc.sync.dma_start(b1p_sb, io["b1p"][:, :])
    fc2b_sb = consts.tile([P, dc], F32)
    nc.sync.dma_start(fc2b_sb, io["fc2_b"][:, :])
    eps_sb = consts.tile([P, 1], F32)
    nc.vector.memset(eps_sb, EPS)
    zero_sb = consts.tile([P, 1], F32)
    nc.vector.memset(zero_sb, 0.0)
    lpsc_sb = consts.tile([P, 1], F32)
    nc.vector.memset(lpsc_sb, float(np.log(PSC)))

    pw_sb = p_pw.tile([P, dc, D], BF16)

    x1T = p_x1.tile([P, dc, no], F32)
    xown = p_xown.tile([P, dc, no], BF16)
    attn1 = p_attn1.tile([P, dc, 512], BF16)

    p_qkvout = _pool(name="p_qkvout", bufs=1)
    qT = p_qkvout.tile([P, dc, no], BF16, tag="q")
    kT = p_qkvout.tile([P, dc, nt], BF16, tag="k")
    v_ext = p_qkvout.tile([P, mt_n, H, VP], FP8, tag="v")
    nc.vector.memset(v_ext[:, :, :, HD:HD + 1], 1.0)

    # ------- Phase 1+2: per-512-group LN1 stats -> z1 -> q/k/v projections --
    # First tokens first: the group-0 x load gates the whole pipeline, so it
    # is issued before any weight DMA.
    xT_t = io["xT"][:, :, :]
    nc.sync.dma_start(xown[:, :, 0:512], xT_t[:, :, 0:512])

    p_wqkv = _pool(name="p_wqkv", bufs=1)
    wqk_hi = p_wqkv.tile([P, dc, 2 * D], FP8, tag="wqk_hi")
    nc.sync.dma_start(wqk_hi, io["wqk_hi"][:, :, :])
    wqk_lo = p_wqkv.tile([P, dc, 2 * D], FP8, tag="wqk_lo")
    nc.sync.dma_start(wqk_lo, io["wqk_lo"][:, :, :])
    wv_hi = p_wqkv.tile([P, dc, D], FP8, tag="wv_hi")
    nc.sync.dma_start(wv_hi, io["wv_hi"][:, :, :])
    wv_lo = p_wqkv.tile([P, dc, D], FP8, tag="wv_lo")
    nc.sync.dma_start(wv_lo, io["wv_lo"][:, :, :])
    w8 = (wqk_hi, wqk_lo)
    wv8 = (wv_hi, wv_lo)

    p_z1g = _pool(name="p_z1g", bufs=2)
    p_xg = _pool(name="p_xg", bufs=2)
    p_stat = _pool(name="p_stat", bufs=2)
    p_scr = _pool(name="p_scr", bufs=1)
    ps_st = _pool(name="ps_st", bufs=2, space="PSUM")

    for g in range(ngk):
        gsl = slice(g * 512, (g + 1) * 512)
        if g < ngq:
            xg = xown[:, :, gsl]
            if g > 0:
                nc.sync.dma_start(xg, xT_t[:, :, gsl])
        else:
            xg = p_xg.tile([P, dc, 512], BF16, tag="xg", name=f"xg_{g}")
            nc.sync.dma_start(xg, xT_t[:, :, gsl])

        nm, rs = _group_stats(nc, ps_st, p_stat, p_scr, tmps, ones_sb,
                              eps_sb, zero_sb, xg, dc, name=f"s1_{g}")
        z1g = p_z1g.tile([P, dc, 512], FP8, tag="z1", name=f"z1_{g}")
        for c in range(dc):
            # all-bf16 add runs in the DVE 2x mode; fp8 quant dominates error
            t = tmps.tile([P, 512], BF16, tag="lnt1")
            nc.vector.tensor_add(t, xg[:, c], nm)
            nc.vector.tensor_mul(z1g[:, c], t, rs)

        # q/k for this token group (k for all groups, q for own groups)
        for cc in range(2 * dc):
            is_q = cc < dc
            if is_q and g >= ngq:
                continue
            msl = slice(cc * P, (cc + 1) * P)
            ps = ps_mm.tile([P, 512], F32, tag="mm")
            for hl in range(2):
                for j in range(dc // 2):
                    nc.tensor.matmul(
                        ps, w8[hl][:, 2 * j:2 * j + 2, msl],
                        z1g[:, 2 * j:2 * j + 2, :],
                        start=(hl == 0 and j == 0),
                        stop=(hl == 1 and j == dc // 2 - 1),
                        perf_mode=DR)
            dst = qT if is_q else kT
            dcc = cc if is_q else cc - dc
            if with_qk_bias:
                nc.scalar.activation(dst[:, dcc, gsl], ps, AF.Identity,
                                     bias=qkb_sb[:, cc:cc + 1],
                                     scale=1.0 / WS)
            else:
                nc.scalar.mul(dst[:, dcc, gsl], ps, 1.0 / WS)

        # v for the 4 token blocks of this group
        for mt in range(4 * g, 4 * g + 4):
            lsl = slice((mt - 4 * g) * P, (mt - 4 * g + 1) * P)
            for half in range(2):
                rhs_sl = slice(half * 384, (half + 1) * 384)
                ps = ps_mm.tile([P, 384], F32, tag="mm")
                for hl in range(2):
                    for j in range(dc // 2):
                        nc.tensor.matmul(
                            ps, z1g[:, 2 * j:2 * j + 2, lsl],
                            wv8[hl][:, 2 * j:2 * j + 2, rhs_sl],
                            start=(hl == 0 and j == 0),
                            stop=(hl == 1 and j == dc // 2 - 1),
                            perf_mode=DR)
                dst = v_ext[:, mt, half * 6:(half + 1) * 6, 0:HD]
                nc.vector.tensor_scalar_mul(
                    dst, ps.rearrange("p (h d) -> p h d", d=HD), 1.0 / WS)

    _rel(ps_st)
    _rel(p_scr)
    _rel(p_stat)
    _rel(p_xg)
    _rel(p_z1g)
    _rel(p_wqkv)
    nc.sync.dma_start(pw_sb, io["pwT"][:, :, :])
    if upto <= 2:
        _cut()
        return

    # ---------------- Phase 3: attention + interleaved MLP(ng0) -------------
    p_win = _pool(name="p_win", bufs=1)     # ng0-scoped mlp buffers
    attn0 = p_win.tile([P, dc, 512], BF16, tag="attn0")
    z2g0 = p_win.tile([P, dc, 512], BF16, tag="z2g0")
    hT0 = p_win.tile([P, hc, 512], BF16, tag="hT0")
    p_stat2 = _pool(name="p_stat2", bufs=2)
    p_x2 = _pool(name="p_x2", bufs=3)
    ps_sc = _pool(name="ps_sc", bufs=2, space="PSUM")
    ps_av = _pool(name="ps_av", bufs=2, space="PSUM")
    p_pT = _pool(name="p_pT", bufs=3)
    p_pair = _pool(name="p_pair", bufs=2)

    w1_t = io["w1T"][:, :, :]
    w2_t = io["w2T"][:, :, :]
    outT_t = io["outT"][:, :, :]
    w1_sl = {}
    w2_sl = {}

    def w1_slice(i):
        if i not in w1_sl:
            w = p_w1.tile([P, dc, 512], BF16, tag="w1", name=f"w1_{i}")
            nc.sync.dma_start(w, w1_t[:, :, i * 512:(i + 1) * 512])
            w1_sl[i] = w
        return w1_sl[i]

    def w2_slice(i):
        if i not in w2_sl:
            w = p_w2.tile([P, hc, P], BF16, tag="w2", name=f"w2_{i}")
            nc.sync.dma_start(w, w2_t[:, :, i * P:(i + 1) * P])
            w2_sl[i] = w
        return w2_sl[i]

    def mlp_steps(ng, attn_t, z2g, hTg, wtag):
        """Generate the MLP step closures for one 512-query group."""
        sl = slice(ng * 512, (ng + 1) * 512)

        def proj_step(ec0):
            for ec in range(ec0, ec0 + 2):
                ps = ps_mm.tile([P, 512], F32, tag="mm")
                for c in range(dc):
                    nc.tensor.matmul(ps, pw_sb[:, c, ec * P:(ec + 1) * P],
                                     attn_t[:, c], start=(c == 0),
                                     stop=(c == dc - 1))
                # x1 = proj + x + pb   (residual built on the fly)
                nc.vector.scalar_tensor_tensor(
                    x1T[:, ec, sl], ps, pb_sb[:, ec:ec + 1],
                    xown[:, ec, sl], OP.add, OP.add)

        def ln2_step():
            nm, rs = _group_stats(nc, ps_mm, p_stat2, p_win, tmps, ones_sb,
                                  eps_sb, zero_sb, x1T[:, :, sl], dc,
                                  name=f"s2_{wtag}", xb=z2g)
            for c in range(dc):
                t = tmps.tile([P, 512], F32, tag="lnt")
                nc.vector.tensor_add(t, x1T[:, c, sl], nm)
                nc.vector.tensor_mul(z2g[:, c], t, rs)

        def fc1_step(cc0, n):
            # stage bias-added pre-activation on DVE, then one big in-place
            # gelu: a single ACT instruction can't be interleaved with
            # softmax exps, so the gelu table loads stay rare.
            for cc in range(cc0, cc0 + n):
                w = w1_slice(cc // 4)
                ci = cc % 4
                ps = ps_mm.tile([P, 512], F32, tag="mm")
                for c in range(dc):
                    nc.tensor.matmul(ps, w[:, c, ci * P:(ci + 1) * P],
                                     z2g[:, c], start=(c == 0),
                                     stop=(c == dc - 1))
                nc.vector.tensor_scalar(hTg[:, cc], ps,
                                        b1p_sb[:, cc:cc + 1], None, OP.add)
            nc.scalar.activation(hTg[:, cc0:cc0 + n], hTg[:, cc0:cc0 + n],
                                 AF.Gelu, bias=zero_sb[:, 0:1])

        def fc2_step(ec0, n):
            for ec in range(ec0, ec0 + n):
                w = w2_slice(ec)
                ps = ps_mm.tile([P, 512], F32, tag="mm")
                for c in range(hc):
                    nc.tensor.matmul(ps, w[:, c], hTg[:, c],
                                     start=(c == 0), stop=(c == hc - 1))
                x2 = p_x2.tile([P, 512], F32, tag="x2", bufs=2)
                if with_fc2_bias:
                    nc.vector.scalar_tensor_tensor(
                        x2, ps, fc2b_sb[:, ec:ec + 1], x1T[:, ec, sl],
                        OP.add, OP.add)
                else:
                    nc.vector.tensor_add(x2, ps, x1T[:, ec, sl])
                nc.sync.dma_start(outT_t[:, ec, sl], x2)

        # fc1 in two 12-wide chunks keeps the gelu ops contiguous on ACT
        # (fewer activation-table reloads against softmax's exp).
        return ([lambda e=e: proj_step(e) for e in (0, 2, 4)]
                + [ln2_step]
                + [lambda c=c: fc1_step(c, 12) for c in (0, 12)]
                + [lambda e=e: fc2_step(e, 3) for e in (0, 3)])

    steps0 = mlp_steps(0, attn0, z2g0, hT0, "a")

    for ng in range(ngq):
        sl = slice(ng * 512, (ng + 1) * 512)
        attn_t = attn0 if ng == 0 else attn1
        pair_sb = None
        bc = None
        for h in range(H):
            hp0, sub0 = divmod(h, 2)
            rows = slice(sub0 * HD, (sub0 + 1) * HD)
            po = ps_av.tile([P, 512], F32, tag="av", name=f"po_{ng}_{h}")
            for t in range(mt_n // 2):
                ps_s = ps_sc.tile([P, 2, 512], F32, tag="sc")
                pp = p_pT.tile([P, 2, 512], FP8, tag="pT", bufs=3)
                for i in range(2):
                    mt = 2 * t + i
                    nc.tensor.matmul(ps_s[:, i],
                                     kT[rows, hp0, mt * P:(mt + 1) * P],
                                     qT[rows, hp0, sl])
                # p = exp(s * SCALE) * PSC, fp8 unnormalized
                nc.scalar.activation(pp, ps_s, AF.Exp,
                                     bias=lpsc_sb[:, 0:1], scale=SCALE)
                nc.tensor.matmul(po[0:HD + 1],
                                 v_ext[:, 2 * t:2 * t + 2, h, 0:HD + 1],
                                 pp, start=(t == 0), stop=(t == mt_n // 2 - 1),
                                 perf_mode=DR)
            # evacuate + normalize per head pair
            hp, sub = divmod(h, 2)
            if sub == 0:
                pair_sb = p_pair.tile([P, 512], F32, tag="pair",
                                      name=f"pair_{ng}_{hp}")
                bc = ps_av.tile([P, 512], F32, tag="av", name=f"bc_{ng}_{hp}")
            nc.vector.tensor_copy(pair_sb[sub * HD:(sub + 1) * HD], po[0:HD])
            rec_h = tmps.tile([1, 512], BF16, tag="rec", bufs=4)
            with nc.allow_low_precision(reason="softmax denom recip"):
                nc.vector.reciprocal(rec_h, po[HD:HD + 1])
            nc.tensor.matmul(bc[sub * HD:(sub + 1) * HD],
                             ones_sb[0:1, 0:HD], rec_h)
            if sub == 1:
                nc.vector.tensor_mul(attn_t[:, hp], pair_sb, bc)
            # interleave ng0's MLP into ng1's softmax window
            if ng == 1 and h < len(steps0):
                steps0[h]()
        if ng == 1:
            for s in steps0[H:]:
                s()

    _rel(p_pair)
    _rel(p_pT)
    _rel(ps_av)
    _rel(ps_sc)
    _rel(p_x2)
    _rel(p_stat2)
    _rel(p_win)
    _rel(p_qkvout)
    if upto <= 3:
        _cut()
        return

    # ---------------- Tail: MLP for ng1 -------------------------------------
    p_tail = _pool(name="p_tail", bufs=1)
    z2g1 = p_tail.tile([P, dc, 512], BF16, tag="z2g1")
    hT1 = p_tail.tile([P, hc, 512], BF16, tag="hT1")
    p_stat3 = _pool(name="p_stat3", bufs=2)
    p_x2b = _pool(name="p_x2b", bufs=3)

    # streamed weight slices were rotated during the window; reload fresh
    w1_sl.clear()
    w2_sl.clear()
    steps1 = _tail_steps(nc, ps_mm, p_stat3, p_tail, p_x2b, tmps, ones_sb,
                         eps_sb, zero_sb, pw_sb, pb_sb, b1p_sb, fc2b_sb,
                         x1T, xown, attn1, z2g1, hT1, w1_slice, w2_slice,
                         outT_t, dc, hc, with_fc2_bias)
    for s in steps1:
        s()

    _rel(p_x2b)
    _rel(p_stat3)
    _rel(p_tail)
    _cut()


def _tail_steps(nc, ps_mm, p_stat, p_win, p_x2, tmps, ones_sb, eps_sb,
                zero_sb, pw_sb, pb_sb, b1p_sb, fc2b_sb, x1T, xown, attn_t,
                z2g, hTg, w1_slice, w2_slice, outT_t, dc, hc, with_fc2_bias):
    sl = slice(512, 1024)
    steps = []

    def proj_step(ec0):
        for ec in range(ec0, ec0 + 2):
            ps = ps_mm.tile([P, 512], F32, tag="mm")
            for c in range(dc):
                nc.tensor.matmul(ps, pw_sb[:, c, ec * P:(ec + 1) * P],
                                 attn_t[:, c], start=(c == 0),
                                 stop=(c == dc - 1))
            nc.vector.scalar_tensor_tensor(
                x1T[:, ec, sl], ps, pb_sb[:, ec:ec + 1],
                xown[:, ec, sl], OP.add, OP.add)

    def ln2_step():
        nm, rs = _group_stats(nc, ps_mm, p_stat, p_win, tmps, ones_sb,
                              eps_sb, zero_sb, x1T[:, :, sl], dc,
                              name="s2_b", xb=z2g)
        for c in range(dc):
            t = tmps.tile([P, 512], F32, tag="lnt")
            nc.vector.tensor_add(t, x1T[:, c, sl], nm)
            nc.vector.tensor_mul(z2g[:, c], t, rs)

    def fc1_step(cc0, n):
        for cc in range(cc0, cc0 + n):
            w = w1_slice(cc // 4)
            ci = cc % 4
            ps = ps_mm.tile([P, 512], F32, tag="mm")
            for c in range(dc):
                nc.tensor.matmul(ps, w[:, c, ci * P:(ci + 1) * P],
                                 z2g[:, c], start=(c == 0),
                                 stop=(c == dc - 1))
            nc.vector.tensor_scalar(hTg[:, cc], ps,
                                    b1p_sb[:, cc:cc + 1], None, OP.add)
        nc.scalar.activation(hTg[:, cc0:cc0 + n], hTg[:, cc0:cc0 + n],
                             AF.Gelu, bias=zero_sb[:, 0:1])

    def fc2_step(ec0, n):
        for ec in range(ec0, ec0 + n):
            w = w2_slice(ec)
            ps = ps_mm.tile([P, 512], F32, tag="mm")
            for c in range(hc):
                nc.tensor.matmul(ps, w[:, c], hTg[:, c],
                                 start=(c == 0), stop=(c == hc - 1))
            x2 = p_x2.tile([P, 512], F32, tag="x2", bufs=2)
            if with_fc2_bias:
                nc.vector.scalar_tensor_tensor(
                    x2, ps, fc2b_sb[:, ec:ec + 1], x1T[:, ec, sl],
                    OP.add, OP.add)
            else:
                nc.vector.tensor_add(x2, ps, x1T[:, ec, sl])
            nc.sync.dma_start(outT_t[:, ec, sl], x2)

    steps += [lambda e=e: proj_step(e) for e in (0, 2, 4)]
    steps.append(ln2_step)
    steps += [lambda c=c: fc1_step(c, 12) for c in (0, 12)]
    steps += [lambda e=e: fc2_step(e, 3) for e in (0, 3)]
    return steps


def _group_stats(nc, ps_pool, p_stat, p_sq, tmps, ones_sb, eps_sb, zero_sb,
                 x_g, dc, name, xb=None):
    """-mean and rstd (replicated over partitions) for one 512-token group.
    x_g: [P, dc, 512] bf16 or f32. For f32, a bf16 staging copy (into xb if
    given) feeds the token-sum matmul at 1 cyc/row."""
    is_f32 = x_g.dtype == F32
    # bf16 stats keep the z1/z2 elementwise chain in DVE 2x mode
    nm = p_stat.tile([P, 512], BF16, tag="nm", name=f"nm_{name}")
    rstd = p_stat.tile([P, 512], BF16, tag="rstd", name=f"rs_{name}")
    xsq = p_sq.tile([P, dc, 512], BF16, tag="xsq", name=f"xsq_{name}")
    for c in range(dc):
        nc.scalar.activation(xsq[:, c], x_g[:, c], AF.Square,
                             bias=zero_sb[:, 0:1])
    if is_f32:
        assert xb is not None
        for c in range(dc):
            nc.vector.tensor_copy(xb[:, c], x_g[:, c])
        xs = xb
    else:
        xs = x_g
    ps_s = ps_pool.tile([P, 512], F32, tag="mm")
    for c in range(dc):
        nc.tensor.matmul(ps_s, ones_sb, xs[:, c],
                         start=(c == 0), stop=(c == dc - 1))
    nc.vector.tensor_scalar_mul(nm, ps_s, -1.0 / D)
    ps_q = ps_pool.tile([P, 512], F32, tag="mm")
    for c in range(dc):
        nc.tensor.matmul(ps_q, ones_sb, xsq[:, c],
                         start=(c == 0), stop=(c == dc - 1))
    # var = E[x^2] - mean^2 ; rstd = exp(-ln(var + eps)/2). The ln/exp pair
    # lives in the same ACT table as softmax's exp -> no table reloads.
    var = tmps.tile([P, 512], F32, tag="var", bufs=1)
    nc.vector.tensor_scalar_mul(var, ps_q, 1.0 / D)
    msq = tmps.tile([P, 512], F32, tag="msq", bufs=1)
    nc.vector.tensor_mul(msq, nm, nm)
    nc.vector.tensor_tensor(var, var, msq, OP.subtract)
    lv = tmps.tile([P, 512], F32, tag="lv", bufs=1)
    nc.scalar.activation(lv, var, AF.Ln, bias=eps_sb[:, 0:1])
    nc.scalar.activation(rstd, lv, AF.Exp, bias=zero_sb[:, 0:1], scale=-0.5)
    return nm, rstd


# --------------------------------------------------------------------------
# Host side
# --------------------------------------------------------------------------

_NC_CACHE = {}


def _get_nc(nt, no, with_qk_bias, with_fc2_bias, reps=1, upto=99):
    key = (nt, no, with_qk_bias, with_fc2_bias, reps, upto)
    if key not in _NC_CACHE:
        _NC_CACHE[key] = _build_nc(nt, no, with_qk_bias, with_fc2_bias, reps,
                                   upto)
    return _NC_CACHE[key]


def _fp8_split(a, s):
    """a: f32 array -> (hi, lo) fp8 pair with hi + lo ~= a * s."""
    hi = np.clip(a * s, -240.0, 240.0).astype(NPFP8)
    lo = np.clip(a * s - hi.astype(np.float32), -240.0, 240.0).astype(NPFP8)
    return hi, lo


def _qk_perm():
    """Head-quad row order: chunk 2g+j holds heads 4g..4g+3, dims j*32..+32,
    with head h on partitions (h%4)*32..(h%4+1)*32."""
    perm = np.empty(2 * D, dtype=np.int64)
    for c in range(DC):
        g, j = divmod(c, 2)
        for p in range(P):
            head = 4 * g + p // 32
            dim = j * 32 + p % 32
            perm[c * P + p] = head * HD + dim
    perm[D:] = perm[:D] + D
    return perm


def _prep_weights(ln1_w, ln1_b, qkv_w, qkv_b, proj_w, proj_b,
                  ln2_w, ln2_b, fc1_w, fc1_b, fc2_w, fc2_b):
    w_qkv = qkv_w * ln1_w[None, :]
    b_qkv = qkv_w @ ln1_b + qkv_b
    pb = proj_b + proj_w @ b_qkv[2 * D:]
    w1 = fc1_w * ln2_w[None, :]
    b1p = fc1_b + fc1_w @ ln2_b

    wqk = w_qkv[:2 * D]
    qkb = b_qkv[:2 * D]

    def col(v, chunks):
        return np.ascontiguousarray(v.reshape(chunks, P).T.astype(np.float32))

    def sb(wT, chunks):
        # [K, M] -> [P, chunks, M] with K = chunks*P (SBUF layout)
        k, m = wT.shape
        return np.ascontiguousarray(
            wT.reshape(chunks, P, m).transpose(1, 0, 2).astype(np.float32))

    wqk_hi, wqk_lo = _fp8_split(sb(wqk.T, DC), WS)
    wv_hi, wv_lo = _fp8_split(sb(w_qkv[2 * D:].T, DC), WS)
    shared = {
        "wqk_hi": wqk_hi, "wqk_lo": wqk_lo,
        "wv_hi": wv_hi, "wv_lo": wv_lo,
        "pwT": sb(proj_w.T, DC).astype(NPBF16),
        "w1T": sb(w1.T, DC).astype(NPBF16),
        "w2T": sb(fc2_w.T, HC).astype(NPBF16),
        "qk_bias": col(qkb, 2 * DC),
        "pb": col(pb, DC),
        "b1p": col(b1p, HC),
        "fc2_b": col(fc2_b, DC),
    }
    flags = (bool(np.any(b_qkv[:2 * D])), bool(np.any(fc2_b)))
    return shared, flags


def build_in_maps(inputs):
    x = np.asarray(inputs["x"], dtype=np.float32)
    args = {k: np.asarray(v, dtype=np.float32) for k, v in inputs.items()
            if k != "x"}
    shared, (f_qk, f_f2) = _prep_weights(
        args["ln1_w"], args["ln1_b"], args["qkv_w"], args["qkv_b"],
        args["proj_w"], args["proj_b"], args["ln2_w"], args["ln2_b"],
        args["fc1_w"], args["fc1_b"], args["fc2_w"], args["fc2_b"])
    no = x.shape[1] // 2
    in_maps = []
    for core in range(N_CORES):
        b, g = divmod(core, 2)
        xr = np.roll(x[b], -g * no, axis=0)
        m = dict(shared)
        m["xT"] = np.ascontiguousarray(
            xr.T.reshape(DC, P, x.shape[1]).transpose(1, 0, 2)).astype(NPBF16)
        in_maps.append(m)
    return in_maps, (f_qk, f_f2)


def run_on_device(inputs, trace=False):
    x = np.asarray(inputs["x"], dtype=np.float32)
    nb, nt, d = x.shape
    no = nt // 2
    in_maps, (f_qk, f_f2) = build_in_maps(inputs)
    nc = _get_nc(nt, no, f_qk, f_f2)

    res = run_bass_kernel_spmd(nc, in_maps, core_ids=list(range(N_CORES)),
                               trace=trace)
    out = np.empty((nb, nt, d), dtype=np.float32)
    for core in range(N_CORES):
        b, g = divmod(core, 2)
        o = res.results[core]["outT"]          # [P, DC, no]
        out[b, g * no:(g + 1) * no, :] = o.transpose(1, 0, 2).reshape(d, no).T
    return out, res


def kernel(**inputs) -> np.ndarray:
    out, _ = run_on_device(inputs, trace=False)
    return out
